# revision 10
# baseline (speedup 1.0000x reference)
"""Trainium2 Bass kernel for nn_CenterDistLoss (segment_reduce).

Strategy (data-parallel over batch, 4 batches per core on 8 cores):
  Per [128, 1024] image tile (rows on partitions):
    labels = RNE-round(y_pr * mask) as bf16      (DVE TT mult + DVE two-op
                                                  tensor_scalar +2^23/-2^23)
    for l in 1..27:
      E_l = is_equal(labels, l)  with accum_out  -> per-image-row counts
      PE matmul with a one-hot-column weight spreads column-sums of E_l
      into PSUM partition-row l, accumulated over the 8 row-blocks.
  Device emits tiny tables: rowacc[128, B_loc, 8, 28] (per-row counts) and
  colcnt[B_loc, 28, 1024] (per-column counts). Host reduces those ~1MB
  tables to centroids and the scalar loss (exact mirror of the reference).
"""

import numpy as np

try:
    import concourse.bass as bass
except ImportError:  # grading env may not have trn_rl_repo on sys.path
    import sys

    sys.path.insert(0, "/opt/trn_rl_repo")
    import concourse.bass as bass

import concourse.bacc as bacc

import concourse.mybir as mybir
from concourse.tile import TileContext
from concourse.bass_utils import run_bass_kernel_spmd
from contextlib import ExitStack

fp32 = mybir.dt.float32
bf16 = mybir.dt.bfloat16

B, H, W = 32, 1024, 1024
N_CORES = 8
B_LOC = B // N_CORES  # 4 batches per core
P = 128
RB = H // P  # 8 row blocks
NL = 28  # label slots 0..27; only 1..27 computed
CB = 2  # column blocks of 512 for PSUM-bank-sized matmuls
MAGIC = float(2**23)

L = 64  # reference label-table size


def _mean_dist_table():
    md = np.full(L, 14.0, dtype=np.float32)
    dists = {2: 18, 3: 18, 4: 18.5, 5: 19, 6: 19.5, 7: 20, 8: 20, 9: 20,
             10: 20.5, 11: 21, 12: 21.5, 13: 22, 14: 22.5, 15: 23, 16: 24.5,
             17: 24.5, 18: 26.5, 19: 28.5, 20: 29.5, 21: 33, 22: 33, 23: 33,
             24: 33, 25: 33, 26: 33}
    for k, v in dists.items():
        md[k] = v
    md[27:] = 30.0
    return md


MEAN_DIST = _mean_dist_table()


def build_nc() -> bass.Bass:
    # Bacc (not raw Bass): it lowers multi-wait instructions through
    # NOP/EventSemaphore passes; raw Bass trips walrus's per-encoding
    # "Too many sync wait commands" limit under Tile's auto-sems.
    nc = bacc.Bacc(trn_type="TRN2")
    y = nc.dram_tensor("y", [B_LOC, H, W], fp32, kind="ExternalInput")
    m = nc.dram_tensor("m", [B_LOC, H, W], fp32, kind="ExternalInput")
    rowacc_out = nc.dram_tensor(
        "rowacc", [P, B_LOC, RB, NL], fp32, kind="ExternalOutput"
    )
    colcnt_out = nc.dram_tensor("colcnt", [B_LOC, NL, W], fp32, kind="ExternalOutput")

    with TileContext(nc) as tc, ExitStack() as ctx:
        io = ctx.enter_context(tc.tile_pool(name="io", bufs=4))
        work = ctx.enter_context(tc.tile_pool(name="work", bufs=3))
        epool = ctx.enter_context(tc.tile_pool(name="epool", bufs=4))
        cpool = ctx.enter_context(tc.tile_pool(name="cpool", bufs=1))
        psum = ctx.enter_context(tc.tile_pool(name="psum", bufs=1, space="PSUM"))

        # One-hot-column weights: wts[:, l, m] = 1.0 iff m == l.
        wts = cpool.tile([P, NL, NL], bf16, name="wts")
        nc.vector.memset(wts[:], 0.0)
        for l in range(1, NL):
            nc.vector.memset(wts[:, l, l : l + 1], 1.0)

        rowacc = cpool.tile([P, B_LOC, RB, NL], fp32, name="rowacc")
        nc.vector.memset(rowacc[:], 0.0)

        ps = [
            [psum.tile([P, W // CB], fp32, name=f"ps_{b}_{cb}") for cb in range(CB)]
            for b in range(B_LOC)
        ]

        for b in range(B_LOC):
            for r in range(RB):
                ytile = io.tile([P, W], fp32, name="ytile", tag="ytile")
                mtile = io.tile([P, W], fp32, name="mtile", tag="mtile")
                nc.sync.dma_start(ytile[:], y[b, r * P : (r + 1) * P, :])
                nc.sync.dma_start(mtile[:], m[b, r * P : (r + 1) * P, :])
                # round(y)*m == round(y*m) for a 0/1 mask; rounding first keeps
                # each instruction to a single cross-engine (DMA) wait.
                ry = work.tile([P, W], fp32, name="ry", tag="ry")
                nc.vector.tensor_scalar(
                    ry[:],
                    ytile[:],
                    MAGIC,
                    MAGIC,
                    mybir.AluOpType.add,
                    mybir.AluOpType.subtract,
                )
                lab = work.tile([P, W], bf16, name="lab", tag="lab")
                nc.vector.tensor_tensor(
                    lab[:], ry[:], mtile[:], mybir.AluOpType.mult
                )
                for l in range(1, NL):
                    e = epool.tile([P, W], bf16, name="e", tag="e")
                    nc.vector.tensor_scalar(
                        e[:],
                        lab[:],
                        float(l),
                        None,
                        mybir.AluOpType.is_equal,
                        mybir.AluOpType.add,
                        accum_out=rowacc[:, b, r, l : l + 1],
                    )
                    for cb in range(CB):
                        nc.tensor.matmul(
                            ps[b][cb][0:NL, :],
                            wts[:, l, :],
                            e[:, cb * (W // CB) : (cb + 1) * (W // CB)],
                            start=(r == 0 and l == 1),
                            stop=(r == RB - 1 and l == NL - 1),
                        )
        for b in range(B_LOC):
            for cb in range(CB):
                drain = work.tile([NL, W // CB], fp32, name="drain", tag="drain")
                nc.vector.tensor_copy(drain[:], ps[b][cb][0:NL, :])
                nc.sync.dma_start(
                    colcnt_out[b, :, cb * (W // CB) : (cb + 1) * (W // CB)], drain[:]
                )
        nc.sync.dma_start(rowacc_out[:], rowacc[:])
    nc.finalize()
    return nc


_NC = None


def _get_nc():
    global _NC
    if _NC is None:
        _NC = build_nc()
    return _NC


def finalize(rowaccs, colcnts):
    """Reduce per-core tables to the scalar loss (mirrors the reference)."""
    counts = np.zeros((B, L), np.float64)
    ysum = np.zeros((B, L), np.float64)
    xsum = np.zeros((B, L), np.float64)
    warange = np.arange(W, dtype=np.float64)
    hidx = (
        np.arange(RB, dtype=np.float64)[None, :, None] * P
        + np.arange(P, dtype=np.float64)[:, None, None]
    )  # [P, RB, 1]
    for c in range(N_CORES):
        rowacc = rowaccs[c].astype(np.float64)  # [P, B_LOC, RB, NL]
        colcnt = colcnts[c].astype(np.float64)  # [B_LOC, NL, W]
        for bl in range(B_LOC):
            b = c * B_LOC + bl
            counts[b, :NL] = colcnt[bl].sum(-1)
            xsum[b, :NL] = (colcnt[bl] * warange[None, :]).sum(-1)
            ysum[b, :NL] = (rowacc[:, bl] * hidx).sum((0, 1))
    safe = np.maximum(counts, 1.0)
    yc = ysum / safe
    xc = xsum / safe
    present = counts > 0
    present[:, 0] = False
    pair_ok = present[:, 1:] & present[:, :-1]
    dist = np.sqrt((xc[:, 1:] - xc[:, :-1]) ** 2 + (yc[:, 1:] - yc[:, :-1]) ** 2)
    loss = np.where(pair_ok, np.abs(dist - MEAN_DIST[1:][None, :]), 0.0).sum()
    return np.float32(loss)


def kernel(y_pr: np.ndarray, mask: np.ndarray, _trace=False, _trace_kwargs=None):
    y = np.ascontiguousarray(
        np.asarray(y_pr, dtype=np.float32).reshape(B, H, W)
    )
    m = np.ascontiguousarray(np.asarray(mask, dtype=np.float32))
    nc = _get_nc()
    in_maps = [
        {"y": y[c * B_LOC : (c + 1) * B_LOC], "m": m[c * B_LOC : (c + 1) * B_LOC]}
        for c in range(N_CORES)
    ]
    kw = {}
    if _trace:
        kw["trace"] = True
        kw.update(_trace_kwargs or {})
    res = run_bass_kernel_spmd(nc, in_maps, core_ids=list(range(N_CORES)), **kw)
    loss = finalize(
        [r["rowacc"] for r in res.results], [r["colcnt"] for r in res.results]
    )
    if _trace:
        return loss, res
    return loss


# revision 14
# speedup vs baseline: 2.0585x; 2.0585x over previous
"""Trainium2 Bass kernel for nn_CenterDistLoss (segment_reduce).

Strategy (data-parallel over batch, 4 batches per core on 8 cores):
  Tiles are [128 partitions, 4096] = image row-block r across all 4 local
  batches (1024 columns each).
    labels = RNE-round(y_pr) * mask  as bf16   (round via +2^23-2^23 two-op
             tensor_scalar, then one TensorTensor multiply)
    for l in 1..27:
      E_l = is_equal(labels, l) -> fp16        (plain tensor_scalar, 4x mode)
      per batch: PE matmul E_l against host-built weights W[l,r] (fp16,
      column l = ones, column 32+l = global row index 128r+p, exact in fp16)
      accumulating into a per-batch [64, 1024] PSUM:
        row l     = per-image-column count of label l
        row 32+l  = per-image-column sum of row-index over label-l pixels
  Host reduces the tiny [4, 64, 1024] tables to centroids and the scalar
  loss (exact mirror of the reference).
"""

import numpy as np

try:
    import concourse.bass as bass
except ImportError:  # grading env may not have trn_rl_repo on sys.path
    import sys

    sys.path.insert(0, "/opt/trn_rl_repo")
    import concourse.bass as bass

import concourse.bacc as bacc
import concourse.mybir as mybir
from concourse.tile import TileContext
from concourse.bass_utils import run_bass_kernel_spmd
from contextlib import ExitStack

fp32 = mybir.dt.float32
bf16 = mybir.dt.bfloat16
fp16 = mybir.dt.float16

B, H, W = 32, 1024, 1024
N_CORES = 8
B_LOC = B // N_CORES  # 4 batches per core
P = 128
RB = H // P  # 8 row blocks
NL = 28  # label slots 0..27; only 1..27 computed
M = 64  # PSUM output partitions: rows 1..27 counts, 33..59 h-sums
FW = B_LOC * W  # free width of a work tile
MAGIC = float(2**23)

L = 64  # reference label-table size


def _mean_dist_table():
    md = np.full(L, 14.0, dtype=np.float32)
    dists = {2: 18, 3: 18, 4: 18.5, 5: 19, 6: 19.5, 7: 20, 8: 20, 9: 20,
             10: 20.5, 11: 21, 12: 21.5, 13: 22, 14: 22.5, 15: 23, 16: 24.5,
             17: 24.5, 18: 26.5, 19: 28.5, 20: 29.5, 21: 33, 22: 33, 23: 33,
             24: 33, 25: 33, 26: 33}
    for k, v in dists.items():
        md[k] = v
    md[27:] = 30.0
    return md


MEAN_DIST = _mean_dist_table()


def build_weights() -> np.ndarray:
    """W[l-1, r, k, m]: column l = 1.0, column 32+l = 128*r + k (fp16-exact)."""
    wts = np.zeros((NL - 1, RB, P, M), np.float16)
    k = np.arange(P, dtype=np.float16)
    for li, l in enumerate(range(1, NL)):
        for r in range(RB):
            wts[li, r, :, l] = 1.0
            wts[li, r, :, 32 + l] = (128 * r + k).astype(np.float16)
    return wts


def build_nc() -> bass.Bass:
    # Bacc (not raw Bass): it lowers multi-wait instructions through
    # NOP/EventSemaphore passes; raw Bass trips walrus's per-encoding
    # "Too many sync wait commands" limit under Tile's auto-sems.
    nc = bacc.Bacc(trn_type="TRN2")
    y = nc.dram_tensor("y", [B_LOC, H, W], fp32, kind="ExternalInput")
    m = nc.dram_tensor("m", [B_LOC, H, W], fp32, kind="ExternalInput")
    wc = nc.dram_tensor("wc", [NL - 1, RB, P, M], fp16, kind="ExternalInput")
    col_out = nc.dram_tensor("colfull", [B_LOC, M, W], fp32, kind="ExternalOutput")

    with TileContext(nc) as tc, ExitStack() as ctx:
        io = ctx.enter_context(tc.tile_pool(name="io", bufs=2))
        work = ctx.enter_context(tc.tile_pool(name="work", bufs=2))
        epool = ctx.enter_context(tc.tile_pool(name="epool", bufs=4))
        cpool = ctx.enter_context(tc.tile_pool(name="cpool", bufs=1))
        psum = ctx.enter_context(tc.tile_pool(name="psum", bufs=1, space="PSUM"))

        wts = cpool.tile([P, NL - 1, RB, M], fp16, name="wts")
        nc.sync.dma_start(wts[:], wc.rearrange("l r k m -> k l r m"))

        ps = [
            [psum.tile([M, W // 2], fp32, name=f"ps_{b}_{cb}") for cb in range(2)]
            for b in range(B_LOC)
        ]

        for r in range(RB):
            ytile = io.tile([P, FW], fp32, name="ytile", tag="ytile")
            mtile = io.tile([P, FW], fp32, name="mtile", tag="mtile")
            # One strided DMA per input: partition = row within block,
            # free = (batch, column).
            nc.sync.dma_start(
                ytile[:], y[:, r * P : (r + 1) * P, :].rearrange("b p w -> p b w")
            )
            nc.sync.dma_start(
                mtile[:], m[:, r * P : (r + 1) * P, :].rearrange("b p w -> p b w")
            )
            ry = work.tile([P, FW], fp32, name="ry", tag="ry")
            nc.vector.tensor_scalar(
                ry[:],
                ytile[:],
                MAGIC,
                MAGIC,
                mybir.AluOpType.add,
                mybir.AluOpType.subtract,
            )
            lab = work.tile([P, FW], bf16, name="lab", tag="lab")
            nc.vector.tensor_tensor(lab[:], ry[:], mtile[:], mybir.AluOpType.mult)
            for li, l in enumerate(range(1, NL)):
                e = epool.tile([P, FW], fp16, name="e", tag="e")
                nc.vector.tensor_scalar(
                    e[:], lab[:], float(l), None, mybir.AluOpType.is_equal
                )
                for b in range(B_LOC):
                    for cb in range(2):
                        nc.tensor.matmul(
                            ps[b][cb][:, :],
                            wts[:, li, r, :],
                            e[:, b * W + cb * (W // 2) : b * W + (cb + 1) * (W // 2)],
                            start=(r == 0 and l == 1),
                            stop=(r == RB - 1 and l == NL - 1),
                        )
        for b in range(B_LOC):
            for cb in range(2):
                drain = work.tile([M, W // 2], fp32, name="drain", tag="drain")
                nc.vector.tensor_copy(drain[:], ps[b][cb][:, :])
                nc.sync.dma_start(
                    col_out[b, :, cb * (W // 2) : (cb + 1) * (W // 2)], drain[:]
                )
    nc.finalize()
    return nc


_NC = None


def _get_nc():
    global _NC
    if _NC is None:
        _NC = build_nc()
    return _NC


def finalize(colfulls):
    """Reduce per-core tables to the scalar loss (mirrors the reference)."""
    counts = np.zeros((B, L), np.float64)
    ysum = np.zeros((B, L), np.float64)
    xsum = np.zeros((B, L), np.float64)
    warange = np.arange(W, dtype=np.float64)
    for c in range(N_CORES):
        cf = colfulls[c].astype(np.float64)  # [B_LOC, M, W]
        for bl in range(B_LOC):
            b = c * B_LOC + bl
            counts[b, 1:NL] = cf[bl, 1:NL].sum(-1)
            xsum[b, 1:NL] = (cf[bl, 1:NL] * warange[None, :]).sum(-1)
            ysum[b, 1:NL] = cf[bl, 33 : 32 + NL].sum(-1)
    safe = np.maximum(counts, 1.0)
    yc = ysum / safe
    xc = xsum / safe
    present = counts > 0
    present[:, 0] = False
    pair_ok = present[:, 1:] & present[:, :-1]
    dist = np.sqrt((xc[:, 1:] - xc[:, :-1]) ** 2 + (yc[:, 1:] - yc[:, :-1]) ** 2)
    loss = np.where(pair_ok, np.abs(dist - MEAN_DIST[1:][None, :]), 0.0).sum()
    return np.float32(loss)


_WC = None


def kernel(y_pr: np.ndarray, mask: np.ndarray, _trace=False, _trace_kwargs=None):
    global _WC
    y = np.ascontiguousarray(np.asarray(y_pr, dtype=np.float32).reshape(B, H, W))
    m = np.ascontiguousarray(np.asarray(mask, dtype=np.float32))
    if _WC is None:
        _WC = build_weights()
    nc = _get_nc()
    in_maps = [
        {
            "y": y[c * B_LOC : (c + 1) * B_LOC],
            "m": m[c * B_LOC : (c + 1) * B_LOC],
            "wc": _WC,
        }
        for c in range(N_CORES)
    ]
    kw = {}
    if _trace:
        kw["trace"] = True
        kw.update(_trace_kwargs or {})
    res = run_bass_kernel_spmd(nc, in_maps, core_ids=list(range(N_CORES)), **kw)
    loss = finalize([r["colfull"] for r in res.results])
    if _trace:
        return loss, res
    return loss


# revision 16
# speedup vs baseline: 2.3943x; 1.1631x over previous
"""Trainium2 Bass kernel for nn_CenterDistLoss (segment_reduce).

Strategy (data-parallel over batch, 4 batches per core on 8 cores):
  Tiles are [128 partitions, 4096] = image row-block r across all 4 local
  batches (1024 columns each).
    labels = RNE-round(y_pr) * mask  as bf16   (round via +2^23-2^23 two-op
             tensor_scalar, then one TensorTensor multiply)
    for l in 1..27:
      E_l = is_equal(labels, l) -> fp16        (plain tensor_scalar, 4x mode)
      per batch: PE matmul E_l against host-built weights W[l,r] (fp16,
      column l = ones, column 32+l = global row index 128r+p, exact in fp16)
      accumulating into a per-batch [64, 1024] PSUM:
        row l     = per-image-column count of label l
        row 32+l  = per-image-column sum of row-index over label-l pixels
  Host reduces the tiny [4, 64, 1024] tables to centroids and the scalar
  loss (exact mirror of the reference).
"""

import numpy as np

try:
    import concourse.bass as bass
except ImportError:  # grading env may not have trn_rl_repo on sys.path
    import sys

    sys.path.insert(0, "/opt/trn_rl_repo")
    import concourse.bass as bass

import concourse.bacc as bacc
import concourse.mybir as mybir
from concourse.tile import TileContext
from concourse.bass_utils import run_bass_kernel_spmd
from contextlib import ExitStack

fp32 = mybir.dt.float32
bf16 = mybir.dt.bfloat16
fp16 = mybir.dt.float16

B, H, W = 32, 1024, 1024
N_CORES = 8
B_LOC = B // N_CORES  # 4 batches per core
P = 128
RB = H // P  # 8 row blocks
NL = 28  # label slots 0..27; only 1..27 computed
M = 96  # PSUM rows: l=counts, 32+l=p-weighted sums, 64+l=128r-weighted counts
FW = B_LOC * W  # free width of a work tile
MAGIC = float(2**23)

L = 64  # reference label-table size


def _mean_dist_table():
    md = np.full(L, 14.0, dtype=np.float32)
    dists = {2: 18, 3: 18, 4: 18.5, 5: 19, 6: 19.5, 7: 20, 8: 20, 9: 20,
             10: 20.5, 11: 21, 12: 21.5, 13: 22, 14: 22.5, 15: 23, 16: 24.5,
             17: 24.5, 18: 26.5, 19: 28.5, 20: 29.5, 21: 33, 22: 33, 23: 33,
             24: 33, 25: 33, 26: 33}
    for k, v in dists.items():
        md[k] = v
    md[27:] = 30.0
    return md


MEAN_DIST = _mean_dist_table()


def build_weights() -> np.ndarray:
    """W[l-1, r, k, m] in bf16 (all values exactly representable):
    column l = 1.0, column 32+l = k (0..127), column 64+l = 128*r."""
    import ml_dtypes

    wts = np.zeros((NL - 1, RB, P, M), ml_dtypes.bfloat16)
    k = np.arange(P, dtype=np.float32)
    for li, l in enumerate(range(1, NL)):
        for r in range(RB):
            wts[li, r, :, l] = 1.0
            wts[li, r, :, 32 + l] = k
            wts[li, r, :, 64 + l] = float(128 * r)
    return wts


def build_nc() -> bass.Bass:
    # Bacc (not raw Bass): it lowers multi-wait instructions through
    # NOP/EventSemaphore passes; raw Bass trips walrus's per-encoding
    # "Too many sync wait commands" limit under Tile's auto-sems.
    nc = bacc.Bacc(trn_type="TRN2")
    y = nc.dram_tensor("y", [B_LOC, H, W], fp32, kind="ExternalInput")
    m = nc.dram_tensor("m", [B_LOC, H, W], fp32, kind="ExternalInput")
    wc = nc.dram_tensor("wc", [NL - 1, RB, P, M], bf16, kind="ExternalInput")
    col_out = nc.dram_tensor("colfull", [B_LOC, M, W], fp32, kind="ExternalOutput")

    with TileContext(nc) as tc, ExitStack() as ctx:
        io = ctx.enter_context(tc.tile_pool(name="io", bufs=2))
        work = ctx.enter_context(tc.tile_pool(name="work", bufs=2))
        epool = ctx.enter_context(tc.tile_pool(name="epool", bufs=4))
        cpool = ctx.enter_context(tc.tile_pool(name="cpool", bufs=1))
        psum = ctx.enter_context(tc.tile_pool(name="psum", bufs=1, space="PSUM"))

        wts = cpool.tile([P, NL - 1, RB, M], bf16, name="wts")
        nc.sync.dma_start(wts[:], wc.rearrange("l r k m -> k l r m"))

        ps = [
            [psum.tile([M, W // 2], fp32, name=f"ps_{b}_{cb}") for cb in range(2)]
            for b in range(B_LOC)
        ]

        for r in range(RB):
            ytile = io.tile([P, FW], fp32, name="ytile", tag="ytile")
            mtile = io.tile([P, FW], fp32, name="mtile", tag="mtile")
            # One strided DMA per input: partition = row within block,
            # free = (batch, column).
            nc.sync.dma_start(
                ytile[:], y[:, r * P : (r + 1) * P, :].rearrange("b p w -> p b w")
            )
            nc.sync.dma_start(
                mtile[:], m[:, r * P : (r + 1) * P, :].rearrange("b p w -> p b w")
            )
            ry = work.tile([P, FW], fp32, name="ry", tag="ry")
            nc.vector.tensor_scalar(
                ry[:],
                ytile[:],
                MAGIC,
                MAGIC,
                mybir.AluOpType.add,
                mybir.AluOpType.subtract,
            )
            lab = work.tile([P, FW], bf16, name="lab", tag="lab")
            nc.vector.tensor_tensor(lab[:], ry[:], mtile[:], mybir.AluOpType.mult)
            for li, l in enumerate(range(1, NL)):
                e = epool.tile([P, FW], bf16, name="e", tag="e")
                nc.vector.tensor_scalar(
                    e[:], lab[:], float(l), None, mybir.AluOpType.is_equal
                )
                for b in range(B_LOC):
                    for cb in range(2):
                        nc.tensor.matmul(
                            ps[b][cb][:, :],
                            wts[:, li, r, :],
                            e[:, b * W + cb * (W // 2) : b * W + (cb + 1) * (W // 2)],
                            start=(r == 0 and l == 1),
                            stop=(r == RB - 1 and l == NL - 1),
                        )
        for b in range(B_LOC):
            for cb in range(2):
                drain = work.tile([M, W // 2], fp32, name="drain", tag="drain")
                nc.vector.tensor_copy(drain[:], ps[b][cb][:, :])
                nc.sync.dma_start(
                    col_out[b, :, cb * (W // 2) : (cb + 1) * (W // 2)], drain[:]
                )
    nc.finalize()
    return nc


_NC = None


def _get_nc():
    global _NC
    if _NC is None:
        _NC = build_nc()
    return _NC


def finalize(colfulls):
    """Reduce per-core tables to the scalar loss (mirrors the reference)."""
    counts = np.zeros((B, L), np.float64)
    ysum = np.zeros((B, L), np.float64)
    xsum = np.zeros((B, L), np.float64)
    warange = np.arange(W, dtype=np.float64)
    for c in range(N_CORES):
        cf = colfulls[c].astype(np.float64)  # [B_LOC, M, W]
        for bl in range(B_LOC):
            b = c * B_LOC + bl
            counts[b, 1:NL] = cf[bl, 1:NL].sum(-1)
            xsum[b, 1:NL] = (cf[bl, 1:NL] * warange[None, :]).sum(-1)
            ysum[b, 1:NL] = cf[bl, 33 : 32 + NL].sum(-1) + cf[bl, 65 : 64 + NL].sum(-1)
    safe = np.maximum(counts, 1.0)
    yc = ysum / safe
    xc = xsum / safe
    present = counts > 0
    present[:, 0] = False
    pair_ok = present[:, 1:] & present[:, :-1]
    dist = np.sqrt((xc[:, 1:] - xc[:, :-1]) ** 2 + (yc[:, 1:] - yc[:, :-1]) ** 2)
    loss = np.where(pair_ok, np.abs(dist - MEAN_DIST[1:][None, :]), 0.0).sum()
    return np.float32(loss)


_WC = None


def kernel(y_pr: np.ndarray, mask: np.ndarray, _trace=False, _trace_kwargs=None):
    global _WC
    y = np.ascontiguousarray(np.asarray(y_pr, dtype=np.float32).reshape(B, H, W))
    m = np.ascontiguousarray(np.asarray(mask, dtype=np.float32))
    if _WC is None:
        _WC = build_weights()
    nc = _get_nc()
    in_maps = [
        {
            "y": y[c * B_LOC : (c + 1) * B_LOC],
            "m": m[c * B_LOC : (c + 1) * B_LOC],
            "wc": _WC,
        }
        for c in range(N_CORES)
    ]
    kw = {}
    if _trace:
        kw["trace"] = True
        kw.update(_trace_kwargs or {})
    res = run_bass_kernel_spmd(nc, in_maps, core_ids=list(range(N_CORES)), **kw)
    loss = finalize([r["colfull"] for r in res.results])
    if _trace:
        return loss, res
    return loss
